# revision 1
# baseline (speedup 1.0000x reference)
"""CapsNet-BCL Trainium2 kernel: 8-core SPMD Bass/Tile implementation.

Host algebra: fc1/fc2 have no nonlinearity between them, so
Weff[t] = fc2_w[t] @ fc1_w[t], beff[t] = fc2_w[t]@fc1_b[t]+fc2_b[t] and
h2 = x @ Weff[t].T + beff[t].  Only tasks r <= eval_t route (softmax mask
-10000 underflows to exactly 0 in fp32), so only route_weights[:, :eval_t+1]
is read.

Sharding: core k computes h2/sem for batches [8k, 8k+8); sem is AllGathered
in two task chunks ({r0..3} then {r4..}); core c computes priors+routing for
capsule c over all 64 batches.  The torch flat view vote(CAP,B,1,L)->
(B,L,CAP) maps output batch b to vote capsule b//8, so core c's vote is
exactly what output batches [8c,8c+8) need: each core emits its own output
slice, no second collective.

Numerics: the routing softmax saturates (|logits| to ~200, top-2 gaps down
to ~2.5), so priors need ~1e-4 relative accuracy — everything in the priors
path stays f32/f32r.

Perf structure vs the original baseline:
 - phase 1 stays in the matmul's natural [(t,c), token] layout: the squash
   norm over t is a 0/1-selector matmul, the per-(c,token) scale is
   replicated back over t with a second tiny matmul, and sem is written to
   DRAM with contiguous 2KB runs (48 descriptors/write instead of 768 —
   HWDGE descriptor generation was the old phase-1 pacing bottleneck).
 - x loaded token-chunk-major, pipelined with the phase-1 matmuls.
 - rw prefetch emitted on the sync ring after the x loads, so x
   descriptors drain first and rw streams during the AllGather window.
 - ACT ops batched by function (Sqrt/Ln/Exp) to amortize table loads.
 - final-linear bias folded into the matmul as a 9th contraction row;
   output writes split across both HWDGE rings.
"""

import sys

import numpy as np

if "/opt/trn_rl_repo" not in sys.path:
    sys.path.insert(0, "/opt/trn_rl_repo")

NTASKS = 10
CAP = 8
L = 256
D = 768
B = 64
N_CORES = 8
BL = B // N_CORES          # batches per core (8)
TOK = BL * L               # tokens per core (2048)
KT = D // 128              # k tiles over D (6)
IT = (L * CAP) // 128      # i tiles over L*CAP (16)
NT = TOK // 512            # phase-1 moving chunks (4)

_CACHE = {}


def _build(A, use_cc=True):
    """Build the 8-core SPMD Bass program for A = eval_t+1 active tasks."""
    import concourse.bass as bass
    import concourse.tile as tile
    import concourse.mybir as mybir
    from concourse import bacc
    from concourse.tile import add_dep_helper

    f32 = mybir.dt.float32
    f32r = mybir.dt.float32r
    Alu = mybir.AluOpType
    Act = mybir.ActivationFunctionType
    X = mybir.AxisListType.X

    nc = bacc.Bacc("TRN2", target_bir_lowering=False, debug=False,
                   num_devices=N_CORES)

    TC = NTASKS * CAP  # 80
    AC = A * CAP
    NPAIR = (A + 1) // 2   # task-pair transpose tiles
    RLO = min(A, 4)        # tasks in collective chunk 0
    CH = [RLO, A - RLO] if A > RLO else [A]   # tasks per chunk
    CH0 = [0, RLO]

    xT = nc.dram_tensor("xT", [D, TOK], f32r, kind="ExternalInput").ap()
    weffT = nc.dram_tensor("weffT", [D, TC], f32r, kind="ExternalInput").ap()
    beff_col = nc.dram_tensor("beff_col", [TC, 1], f32,
                              kind="ExternalInput").ap()
    # rw_h[p, (r, k, o)] = route_weights[core, r, i2(k,p), o] where the
    # contraction index is reordered to i2 = c*L + l (phase-1 sem rows are
    # (t, c) with token cols, so gathered sem transposes to (c, l) order)
    rw = nc.dram_tensor("rw", [128, A * IT * L], f32r,
                        kind="ExternalInput").ap()
    # wlT9 = [larger_w[e].T; larger_b[e]] -- bias folded in as a 9th
    # contraction row so phase 6 needs no separate bias add
    wlT9 = nc.dram_tensor("wlT9", [CAP + 1, D], f32r,
                          kind="ExternalInput").ap()
    ones_row = nc.dram_tensor("ones_row", [1, 32 * B], f32,
                              kind="ExternalInput").ap()
    # squash helpers: selT[(t,c), c'] = (c == c'); repT[c, (t<A,c')] = (c==c')
    selT = nc.dram_tensor("selT", [TC, CAP], f32r,
                          kind="ExternalInput").ap()
    repT = nc.dram_tensor("repT", [CAP, AC], f32r,
                          kind="ExternalInput").ap()
    ident = nc.dram_tensor("ident", [128, 128], f32, kind="ExternalInput").ap()
    out = nc.dram_tensor("out", [BL, L, D], f32, kind="ExternalOutput").ap()

    # collective chunks by task: rows (t, c) t-major, cols (b_l, l)
    sem_p = [nc.dram_tensor(f"sem_p{i}", [n * CAP, TOK], f32).ap()
             for i, n in enumerate(CH)]
    gath_p = [nc.dram_tensor(f"gath_p{i}", [N_CORES * n * CAP, TOK], f32,
                             addr_space="Shared").ap()
              for i, n in enumerate(CH)]
    # tiny 2-rank collective (single algorithm step) to absorb the
    # first-op ncfw reaction cost while phase 1 is still computing
    cc_warm_in = nc.dram_tensor("cc_warm_in", [1, 16], f32).ap()
    cc_warm_out = nc.dram_tensor("cc_warm_out", [4, 16], f32,
                                 addr_space="Shared").ap()
    voteT_dram = nc.dram_tensor("voteT_dram", [L, B], f32).ap()

    with tile.TileContext(nc) as tc:
        with tc.tile_pool(name="singles", bufs=1) as singles:
            # ---- constants ----
            weff_sb = singles.tile([128, KT * TC], f32r)
            nc.sync.dma_start(out=weff_sb,
                              in_=weffT.rearrange("(k p) c -> p k c", p=128))
            beff_sb = singles.tile([TC, 1], f32)
            nc.sync.dma_start(out=beff_sb, in_=beff_col)
            ident_sb = singles.tile([128, 128], f32)
            nc.sync.dma_start(out=ident_sb, in_=ident)
            wlT_sb = singles.tile([CAP + 1, D], f32r)
            nc.sync.dma_start(out=wlT_sb, in_=wlT9)
            sel_sb = singles.tile([TC, CAP], f32r)
            nc.sync.dma_start(out=sel_sb, in_=selT)
            rep_sb = singles.tile([CAP, AC], f32r)
            nc.sync.dma_start(out=rep_sb, in_=repT)

            priors_sb = singles.tile([64, A * L], f32)
            semT_sb = singles.tile([128, NPAIR * IT * 128], f32r)

            rw_sb = []
            for r in range(A):
                rwt = singles.tile([128, IT * L], f32r, tag=f"rw{r}")
                rw_sb.append(rwt)

            # ===== Phase 1: semantic stage, batch-parallel ================
            # All in the [(t,c), token] layout h2 is produced in:
            #   h2a[80, 512] (+bias, DVE); h2sq = h2a^2 (DVE);
            #   sq[c, tok] = selT.T @ h2sq (PE); scal = sqrt(sq)/(1+sq)
            #   with 1/(1+sq) = exp(-ln(1+sq)) -- ACT ops batched by
            #   function so table reloads (~1.3us each) happen ~3x total;
            #   scal_rep = repT.T @ scal (PE); sem = h2a[:AC] * scal_rep
            #   (DVE) -> contiguous DRAM write (2KB runs).
            with (
                tc.tile_pool(name="x_pool", bufs=8) as xpool,
                tc.tile_pool(name="pA", bufs=2, space="PSUM") as pA,
                tc.tile_pool(name="pS", bufs=4, space="PSUM") as pS,
                tc.tile_pool(name="pR", bufs=2, space="PSUM") as pR,
                tc.tile_pool(name="h2a_pool", bufs=4) as hapool,
                tc.tile_pool(name="sem_pool", bufs=2) as spool,
                tc.tile_pool(name="sq_pool", bufs=4) as qpool,
            ):
                h2as, psqs, rts, dens, lnds, rdens, scals = \
                    [], [], [], [], [], [], []
                for nt in range(NT):            # 4 chunks of 512 tokens
                    xks = []
                    for k in range(KT):
                        xk = xpool.tile([128, 512], f32r, tag="xk")
                        nc.sync.dma_start(
                            out=xk,
                            in_=xT[k * 128:(k + 1) * 128,
                                   nt * 512:(nt + 1) * 512])
                        xks.append(xk)
                    psa = pA.tile([TC, 512], f32, tag="psa")
                    for k in range(KT):
                        nc.tensor.matmul(
                            psa,
                            lhsT=weff_sb[:, k * TC:(k + 1) * TC],
                            rhs=xks[k],
                            start=(k == 0), stop=(k == KT - 1),
                        )
                    h2a = hapool.tile([TC, 512], f32, tag="h2a")
                    nc.vector.tensor_scalar_add(h2a, psa, beff_sb)
                    h2sq = spool.tile([TC, 512], f32r, tag="h2sq")
                    nc.vector.tensor_mul(h2sq, h2a, h2a)
                    psq = pS.tile([CAP, 512], f32, tag="psq")
                    nc.tensor.matmul(psq, lhsT=sel_sb, rhs=h2sq,
                                     start=True, stop=True)
                    h2as.append(h2a)
                    psqs.append(psq)
                for nt in range(NT):
                    rt = qpool.tile([CAP, 512], f32, tag="rt")
                    nc.scalar.activation(rt, psqs[nt], Act.Sqrt)
                    rts.append(rt)
                for nt in range(NT):
                    den = qpool.tile([CAP, 512], f32, tag="den")
                    nc.vector.tensor_scalar_add(den, psqs[nt], 1.0)
                    dens.append(den)
                for nt in range(NT):
                    lnd = qpool.tile([CAP, 512], f32, tag="lnd")
                    nc.scalar.activation(lnd, dens[nt], Act.Ln)
                    lnds.append(lnd)
                for nt in range(NT):
                    rden = qpool.tile([CAP, 512], f32, tag="rden")
                    nc.scalar.activation(rden, lnds[nt], Act.Exp,
                                         scale=-1.0)
                    rdens.append(rden)
                last_sem_write = None
                for nt in range(NT):
                    scal = qpool.tile([CAP, 512], f32r, tag="scal")
                    nc.vector.tensor_mul(scal, rts[nt], rdens[nt])
                    prep = pR.tile([AC, 512], f32, tag="prep")
                    nc.tensor.matmul(prep, lhsT=rep_sb, rhs=scal,
                                     start=True, stop=True)
                    sem = spool.tile([AC, 512], f32, tag="sem")
                    nc.vector.tensor_tensor(out=sem, in0=h2as[nt][:AC],
                                            in1=prep, op=Alu.mult)
                    for i, n in enumerate(CH):
                        wr = nc.sync.dma_start(
                            out=sem_p[i][:, nt * 512:(nt + 1) * 512],
                            in_=sem[CH0[i] * CAP:(CH0[i] + n) * CAP])
                        last_sem_write = wr

            # ---- rw prefetch: emitted on the sync ring AFTER the phase-1
            # x loads, so x descriptors drain first and rw streams during
            # the AllGather window ----
            for r in range(A):
                nc.sync.dma_start(
                    out=rw_sb[r], in_=rw[:, r * IT * L:(r + 1) * IT * L])

            # (no PE keep-warm burn here: measured HAM shows a sustained
            # dummy-matmul burn trips the SW power throttle to k=4/8 right
            # when the priors matmuls need the array)

            # ===== Phase 2: allgather sem (task chunks) ===================
            if use_cc:
                for i in range(len(CH)):
                    nc.gpsimd.collective_compute(
                        "AllGather", Alu.bypass,
                        replica_groups=[list(range(N_CORES))],
                        ins=[sem_p[i][:]], outs=[gath_p[i][:]])
            else:
                for i in range(len(CH)):
                    nc.sync.dma_start(out=gath_p[i][0:CH[i] * CAP],
                                      in_=sem_p[i][:])

            # ===== Phase 3+4: gather-transpose + priors ===================
            # Pair tiles: tile t holds tasks (2t, 2t+1) x all 64 batches,
            # partition = (task, rank, b_l), cols = (c, l); PE transposes
            # give semT[(c,l)-slice, (task, batch)] for the priors lhsT.
            NP = NPAIR

            def chunk_r(r):
                return (0, r) if r < RLO else (1, r - RLO)

            with (
                tc.tile_pool(name="gpool", bufs=2) as gpool,
                tc.tile_pool(name="pT", bufs=4, space="PSUM") as pT,
                tc.tile_pool(name="pP", bufs=3, space="PSUM") as pP,
            ):
                g_tiles = []
                for t in range(NP):
                    g_sb = gpool.tile([128, L * CAP], f32, tag="g")
                    for ri in range(2):
                        if 2 * t + ri >= A:
                            continue
                        ci, rloc = chunk_r(2 * t + ri)
                        for rank in range(N_CORES):
                            base = (rank * CH[ci] + rloc) * CAP
                            eng = nc.sync if (rank % 2 == 0) else nc.scalar
                            eng.dma_start(
                                out=g_sb[ri * 64 + rank * 8:
                                         ri * 64 + rank * 8 + 8].rearrange(
                                    "p (c l) -> p c l", c=CAP),
                                in_=gath_p[ci][base:base + CAP, :].rearrange(
                                    "c (b l) -> b c l", b=BL))
                    g_tiles.append(g_sb)

                for t in range(NP):
                    for k in range(IT):
                        psT = pT.tile([128, 128], f32, tag="psT")
                        nc.tensor.transpose(
                            psT, in_=g_tiles[t][:, k * 128:(k + 1) * 128],
                            identity=ident_sb)
                        cp = nc.vector if (k % 3) else nc.scalar
                        dst = semT_sb[:, (t * IT + k) * 128:
                                      (t * IT + k + 1) * 128]
                        if cp is nc.vector:
                            nc.vector.tensor_copy(out=dst, in_=psT)
                        else:
                            nc.scalar.activation(dst, psT, Act.Copy)
                    for ri in range(2):
                        r = 2 * t + ri
                        if r >= A:
                            continue
                        pp = pP.tile([64, L], f32, tag="pp")
                        for k in range(IT):
                            base = (t * IT + k) * 128 + ri * 64
                            nc.tensor.matmul(
                                pp, lhsT=semT_sb[:, base:base + 64],
                                rhs=rw_sb[r][:, k * L:(k + 1) * L],
                                start=(k == 0), stop=(k == IT - 1))
                        cp = nc.vector if (r % 2 == 0) else nc.scalar
                        dst = priors_sb[:, r * L:(r + 1) * L]
                        if cp is nc.vector:
                            nc.vector.tensor_copy(out=dst, in_=pp)
                        else:
                            nc.scalar.activation(dst, pp, Act.Copy)

            # ===== Phase 5: routing (vectorized over r) ===================
            with (
                tc.tile_pool(name="route", bufs=1) as rp,
                tc.tile_pool(name="pV", bufs=2, space="PSUM") as pV,
            ):
                vote = rp.tile([64, L], f32)
                scr = rp.tile([64, L], f32)
                big = rp.tile([64, A * L], f32)
                l1 = rp.tile([64, A], f32)
                l2 = rp.tile([64, A], f32)
                dots_raw = rp.tile([64, A], f32)
                dots = rp.tile([64, A], f32)
                ex = rp.tile([64, A], f32)
                probs = rp.tile([64, A], f32)
                n2 = rp.tile([64, 1], f32)
                rt2 = rp.tile([64, 1], f32)
                den2 = rp.tile([64, 1], f32)
                rden2 = rp.tile([64, 1], f32)
                sc2 = rp.tile([64, 1], f32)
                mx = rp.tile([64, 1], f32)
                nmx = rp.tile([64, 1], f32)
                ssum = rp.tile([64, 1], f32)
                rsum = rp.tile([64, 1], f32)

                def warm(dep, m):
                    # tiny matmul with a true dep on the routing chain --
                    # keeps the PE HAM un-throttled through phase 5
                    pdum = pV.tile([64, 128], f32, tag="pdum")
                    nc.tensor.matmul(pdum[:m], lhsT=dep[:, 0:m],
                                     rhs=priors_sb[:, 0:128],
                                     start=True, stop=True)

                def squash_scal():
                    # sc2 = sqrt(n2)/(1+n2); outsq = sc2*vote is never
                    # materialized -- dots get scaled by sc2 instead.
                    nc.vector.tensor_mul(scr, vote, vote)
                    nc.vector.tensor_reduce(out=n2, in_=scr, axis=X,
                                            op=Alu.add)
                    nc.scalar.activation(rt2, n2, Act.Sqrt)
                    nc.vector.tensor_scalar_add(den2, n2, 1.0)
                    nc.vector.reciprocal(rden2, den2)
                    nc.vector.tensor_mul(sc2, rt2, rden2)

                def logit_update(l_prev, l_new):
                    for r in range(A):
                        nc.vector.scalar_tensor_tensor(
                            out=big[:, r * L:(r + 1) * L],
                            in0=priors_sb[:, r * L:(r + 1) * L],
                            scalar=1.0, in1=vote,
                            op0=Alu.mult, op1=Alu.mult,
                            accum_out=dots_raw[:, r:r + 1])
                    if l_prev is None:
                        nc.vector.tensor_scalar_mul(l_new, dots_raw, sc2)
                    else:
                        nc.vector.tensor_scalar_mul(dots, dots_raw, sc2)
                        nc.vector.tensor_add(l_new, dots, l_prev)

                def softmax_vote(l_in):
                    nc.vector.tensor_reduce(out=mx, in_=l_in, axis=X,
                                            op=Alu.max)
                    nc.vector.tensor_scalar_mul(nmx, mx, -1.0)
                    nc.scalar.activation(ex, l_in, Act.Exp, bias=nmx,
                                         accum_out=ssum)
                    nc.vector.reciprocal(rsum, ssum)
                    nc.vector.tensor_scalar_mul(probs, ex, rsum)
                    pr_b = bass.AP(
                        tensor=probs.tensor, offset=probs.offset,
                        ap=[probs.ap[0], [1, A], [0, L]])
                    nc.vector.tensor_tensor(
                        out=big.rearrange("p (r o) -> p r o", r=A),
                        in0=priors_sb.rearrange("p (r o) -> p r o", r=A),
                        in1=pr_b, op=Alu.mult)
                    nc.vector.tensor_reduce(
                        out=vote,
                        in_=big.rearrange("p (r o) -> p o r", r=A),
                        axis=X, op=Alu.add)

                # iter 1: uniform probs = 1/A
                nc.vector.tensor_reduce(
                    out=scr,
                    in_=priors_sb.rearrange("p (r o) -> p o r", r=A),
                    axis=X, op=Alu.add)
                nc.vector.tensor_scalar_mul(vote, scr, 1.0 / A)
                squash_scal()
                warm(vote, 64)
                logit_update(None, l1)
                warm(l1, A)
                softmax_vote(l1)
                warm(vote, 64)
                squash_scal()
                logit_update(l1, l2)
                warm(l2, A)
                softmax_vote(l2)

                # transpose vote [64, 256] -> voteT_dram [256, 64]
                vT_sb = rp.tile([128, 128], f32)
                for half in range(2):
                    pv = pV.tile([128, 64], f32, tag="pv")
                    nc.tensor.transpose(
                        pv, in_=vote[:, half * 128:(half + 1) * 128],
                        identity=ident_sb[:64, :64])
                    nc.vector.tensor_copy(
                        out=vT_sb[:, half * 64:(half + 1) * 64], in_=pv)
                    nc.sync.dma_start(
                        out=voteT_dram[half * 128:(half + 1) * 128],
                        in_=vT_sb[:, half * 64:(half + 1) * 64])

            # ===== Phase 6: final linear ==================================
            # voteT_dram[o, b]; h_blT[cap, l] = voteT[(l%32)*8+cap,
            # b_l*8 + l//32].  vt2[cap, (lr, b)] loads with 256B bursts;
            # row CAP is all-ones so wlT9's bias row lands in the matmul.
            with (
                tc.tile_pool(name="vt", bufs=1) as vtp,
                tc.tile_pool(name="pF", bufs=4, space="PSUM") as pF,
                tc.tile_pool(name="outp", bufs=3) as op_,
            ):
                vt2 = vtp.tile([CAP + 1, 32 * B], f32)
                src = bass.AP(
                    tensor=voteT_dram.tensor, offset=voteT_dram.offset,
                    ap=[[B, CAP], [CAP * B, 32], [1, B]])
                nc.sync.dma_start(out=vt2[:CAP], in_=src)
                nc.sync.dma_start(out=vt2[CAP:CAP + 1], in_=ones_row)
                # permute free layout (lr, b) -> (b, lr) during the f32r
                # convert, so each lhsT is a contiguous 128-col slice
                vt2r = vtp.tile([CAP + 1, 32 * B], f32r)
                nc.vector.tensor_copy(
                    out=vt2r.rearrange("p (b lr) -> p b lr", lr=32),
                    in_=vt2.rearrange("p (lr b) -> p b lr", lr=32))
                NH = 2
                for b_l in range(BL):
                    for lt in range(2):
                        o_sb = op_.tile([128, D], f32, tag="o")
                        lhsT = vt2r[:, (b_l * CAP + lt * 4) * 32:
                                    (b_l * CAP + lt * 4) * 32 + 128]
                        for nh in range(NH):
                            pf = pF.tile([128, D // NH], f32, tag="pf")
                            nc.tensor.matmul(
                                pf, lhsT=lhsT,
                                rhs=wlT_sb[:, nh * (D // NH):
                                           (nh + 1) * (D // NH)],
                                start=True, stop=True)
                            dst = o_sb[:, nh * (D // NH):(nh + 1) * (D // NH)]
                            if nh == 0:
                                nc.vector.tensor_copy(out=dst, in_=pf)
                            else:
                                nc.scalar.activation(dst, pf, Act.Copy)
                        eng = nc.sync if (b_l % 2 == 0) else nc.scalar
                        eng.dma_start(
                            out=out[b_l, lt * 128:(lt + 1) * 128, :],
                            in_=o_sb)

    nc.compile()
    return nc


def _host_prep(x, fc1_w, fc1_b, fc2_w, fc2_b, route_weights, larger_w,
               larger_b, eval_t):
    A = int(eval_t) + 1
    f64 = np.float64
    weff = np.einsum("tcd,tdi->tci", fc2_w.astype(f64), fc1_w.astype(f64))
    beff = (np.einsum("tcd,td->tc", fc2_w.astype(f64), fc1_b.astype(f64))
            + fc2_b.astype(f64))
    weffT = np.ascontiguousarray(
        weff.reshape(NTASKS * CAP, D).T).astype(np.float32)
    beff_col = beff.reshape(NTASKS * CAP, 1).astype(np.float32)
    wlT9 = np.ascontiguousarray(np.concatenate(
        [larger_w[int(eval_t)].T, larger_b[int(eval_t)].reshape(1, D)],
        axis=0)).astype(np.float32)
    ones_row = np.ones((1, 32 * B), dtype=np.float32)
    selT = np.tile(np.eye(CAP, dtype=np.float32), (NTASKS, 1))
    repT = np.tile(np.eye(CAP, dtype=np.float32), (1, A))
    ident = np.eye(128, dtype=np.float32)

    in_maps = []
    for c in range(N_CORES):
        xT_c = np.ascontiguousarray(
            x[c * BL:(c + 1) * BL].reshape(TOK, D).T).astype(np.float32)
        # reorder the contraction index to i2 = c2*L + l, then k-tile:
        # rw_c[p, (r, k, o)] = route_weights[c, r, l(k,p)*CAP + c2(k,p), o]
        rw2 = route_weights[c, :A].reshape(A, L, CAP, L).transpose(0, 2, 1, 3)
        rw_c = np.ascontiguousarray(
            rw2.reshape(A, IT, 128, L)
            .transpose(2, 0, 1, 3).reshape(128, A * IT * L)).astype(
                np.float32)
        in_maps.append({
            "xT": xT_c, "weffT": weffT, "beff_col": beff_col, "rw": rw_c,
            "wlT9": wlT9, "ones_row": ones_row, "selT": selT, "repT": repT,
            "ident": ident,
        })
    return A, in_maps


def kernel(**inputs):
    from concourse.bass_utils import run_bass_kernel_spmd

    A, in_maps = _host_prep(**inputs)
    if A not in _CACHE:
        _CACHE[A] = _build(A)
    nc = _CACHE[A]
    res = run_bass_kernel_spmd(nc, in_maps, core_ids=list(range(N_CORES)))
    return np.concatenate(
        [res.results[c]["out"] for c in range(N_CORES)], axis=0)



# revision 15
# speedup vs baseline: 1.1544x; 1.1544x over previous
"""CapsNet-BCL Trainium2 kernel: 8-core SPMD Bass/Tile implementation.

Host algebra: fc1/fc2 have no nonlinearity between them, so
Weff[t] = fc2_w[t] @ fc1_w[t], beff[t] = fc2_w[t]@fc1_b[t]+fc2_b[t] and
h2 = x @ Weff[t].T + beff[t].  Only tasks r <= eval_t route (softmax mask
-10000 underflows to exactly 0 in fp32), so only route_weights[:, :eval_t+1]
is read.

Sharding: core k computes h2/sem for batches [8k, 8k+8); sem is AllGathered
in two l-range chunks (l in [0,128) then [128,256)) so the collective
pipelines with phase 1; core c computes priors+routing for capsule c over
all 64 batches.  The torch flat view vote(CAP,B,1,L)->(B,L,CAP) maps output
batch b to vote capsule b//8, so core c's vote is exactly what output
batches [8c,8c+8) need: each core emits its own output slice.

Perf structure vs the previous baseline (228-265us):
 - phase 1 tokens are l-chunk-major: the sem AllGather for l-chunk 0 fires
   mid-phase-1 instead of after it, hiding most of the collective's
   ring latency behind compute; a tiny warm-up collective at t~0 absorbs
   the ncfw first-op reaction.
 - route_weights stored fp16 in DRAM (numerically validated: 5.5e-3 final
   rel err vs the 2e-2 gate) and upcast to f32r on DVE/ACT during the
   collective wait -- halves the dominant HBM stream; the priors matmul
   stays f32/f32r (sem quantization to fp16 was measured at 1.35e-2 --
   too close to the gate).
 - per-l-chunk gather-transpose + priors matmuls (PSUM accumulation over
   chunks) so only the last chunk's work sits on the tail.
 - routing rewritten with folded scales: squash norms and softmax
   normalizations are never materialized (q = sqrt(n2raw)/(A^2+n2raw)
   style foldings, verified algebraically equal); dot-batches via one
   broadcast multiply + segmented reduce.
 - x loaded in 4 wide DMAs (6 k-tiles each) to cut HWDGE issue time.
 - final-linear bias folded into the matmul as a 9th contraction row;
   output writes split across both HWDGE rings; small true-dep matmuls
   keep the PE HAM un-throttled through routing.
"""

import sys

import numpy as np

if "/opt/trn_rl_repo" not in sys.path:
    sys.path.insert(0, "/opt/trn_rl_repo")

NTASKS = 10
CAP = 8
L = 256
D = 768
B = 64
N_CORES = 8
BL = B // N_CORES          # batches per core (8)
TOK = BL * L               # tokens per core (2048)
KT = D // 128              # k tiles over D (6)
NCH = 2                    # l-chunks (128 l values each)
LCH = L // NCH             # l per chunk (128)
NT = 4                     # phase-1 moving subchunks (512 tokens)

_CACHE = {}


def _build(A):
    """Build the 8-core SPMD Bass program for A = eval_t+1 active tasks."""
    import concourse.bass as bass
    import concourse.tile as tile
    import concourse.mybir as mybir
    from concourse import bacc

    f32 = mybir.dt.float32
    f32r = mybir.dt.float32r
    f16 = mybir.dt.float16
    Alu = mybir.AluOpType
    Act = mybir.ActivationFunctionType
    X = mybir.AxisListType.X

    nc = bacc.Bacc("TRN2", target_bir_lowering=False, debug=False,
                   num_devices=N_CORES)

    TC = NTASKS * CAP  # 80
    AC = A * CAP       # active rows (48)
    NPAIR = (A + 1) // 2

    xT = nc.dram_tensor("xT", [D, TOK], f32r, kind="ExternalInput").ap()
    weffT = nc.dram_tensor("weffT", [D, TC], f32r, kind="ExternalInput").ap()
    beff_col = nc.dram_tensor("beff_col", [TC, 1], f32,
                              kind="ExternalInput").ap()
    # rw16[p, ((r*NCH+k)*CAP+j)*L + o] = route_weights[core, r,
    #   (k*LCH+p)*CAP + j, o]  (fp16; i2 = l*CAP + c' contraction order)
    rw16 = nc.dram_tensor("rw16", [128, A * NCH * CAP * L], f16,
                          kind="ExternalInput").ap()
    # wlT9 = [larger_w[e].T; larger_b[e]] -- bias folded in as a 9th
    # contraction row so phase 6 needs no separate bias add
    wlT9 = nc.dram_tensor("wlT9", [CAP + 1, D], f32r,
                          kind="ExternalInput").ap()
    ones_row = nc.dram_tensor("ones_row", [1, 32 * B], f32,
                              kind="ExternalInput").ap()
    # squash helpers: selT[(t,c), c'] = (c == c'); repT[c, (t<A,c')] = (c==c')
    selT = nc.dram_tensor("selT", [TC, CAP], f32r,
                          kind="ExternalInput").ap()
    repT = nc.dram_tensor("repT", [CAP, AC], f32r,
                          kind="ExternalInput").ap()
    ident = nc.dram_tensor("ident", [128, 128], f32, kind="ExternalInput").ap()
    out = nc.dram_tensor("out", [BL, L, D], f32, kind="ExternalOutput").ap()

    # collective chunks by l-range: rows (b_l, t, c), cols l_in_chunk
    # (b_l outermost so the post-gather read is 1KB-contiguous per batch)
    sem_p = [nc.dram_tensor(f"sem_p{i}", [BL * AC, LCH], f32).ap()
             for i in range(NCH)]
    gath_p = [nc.dram_tensor(f"gath_p{i}", [N_CORES * BL * AC, LCH], f32,
                             addr_space="Shared").ap()
              for i in range(NCH)]
    # tiny collective fired at t~0 with no deps: wakes ncfw / absorbs the
    # first-op reaction cost while phase 1 is still computing
    cc_warm_in = nc.dram_tensor("cc_warm_in", [1, 16], f32).ap()
    cc_warm_out = nc.dram_tensor("cc_warm_out", [N_CORES, 16], f32,
                                 addr_space="Shared").ap()
    voteT_dram = nc.dram_tensor("voteT_dram", [L, B], f32).ap()

    RG = [list(range(N_CORES))]

    with tile.TileContext(nc) as tc:
        with tc.tile_pool(name="singles", bufs=1) as singles:
            # warm-up collective: fires as soon as its 64B input is zeroed
            # (collectives cannot read IO tensors, so memset+DMA first)
            warm_sb = singles.tile([1, 16], f32)
            nc.vector.memset(warm_sb, 0.0)
            nc.gpsimd.dma_start(out=cc_warm_in, in_=warm_sb)
            nc.gpsimd.collective_compute(
                "AllGather", Alu.bypass, replica_groups=RG,
                ins=[cc_warm_in[:]], outs=[cc_warm_out[:]])

            # ---- constants (scalar ring; sync ring is reserved for x) ----
            weff_sb = singles.tile([128, KT * TC], f32r)
            nc.scalar.dma_start(out=weff_sb,
                                in_=weffT.rearrange("(k p) c -> p k c", p=128))
            beff_sb = singles.tile([TC, 1], f32)
            nc.scalar.dma_start(out=beff_sb, in_=beff_col)
            ident_sb = singles.tile([128, 128], f32)
            nc.scalar.dma_start(out=ident_sb, in_=ident)
            wlT_sb = singles.tile([CAP + 1, D], f32r)
            nc.scalar.dma_start(out=wlT_sb, in_=wlT9)
            sel_sb = singles.tile([TC, CAP], f32r)
            nc.scalar.dma_start(out=sel_sb, in_=selT)
            rep_sb = singles.tile([CAP, AC], f32r)
            nc.scalar.dma_start(out=rep_sb, in_=repT)

            priors_sb = singles.tile([64, A * L], f32)
            # semT[(i-slice), (pair block, (ri, rank, b_l))] -- reused
            # across chunks (PE FIFO order makes the WAR free)
            semT_sb = singles.tile([128, NPAIR * CAP * 128], f32r)

            # preload the ACT Sqrt table before phase 1 needs it
            sqrt_warm = singles.tile([1, 16], f32)
            nc.scalar.activation(sqrt_warm, ident_sb[0:1, 0:16], Act.Sqrt)

            # ===== Phase 1: semantic stage, batch-parallel ================
            # cols are (chunk, b_l, l_in_chunk); per 512-col subchunk:
            #   h2a = psa + beff (ACT, bias); h2sq = h2a^2 (DVE);
            #   sq[c, tok] = selT.T @ h2sq (PE); scal = sqrt(sq)/(1+sq)
            #   via ACT Sqrt + DVE reciprocal; scal_rep = repT.T @ scal (PE);
            #   sem = h2a[:AC] * scal_rep (DVE) -> DRAM chunk buffer.
            with (
                tc.tile_pool(name="x_pool", bufs=2) as xpool,
                tc.tile_pool(name="pA", bufs=2, space="PSUM") as pA,
                tc.tile_pool(name="pS", bufs=2, space="PSUM") as pS,
                tc.tile_pool(name="pR", bufs=2, space="PSUM") as pR,
                tc.tile_pool(name="h2a_pool", bufs=2) as hapool,
                tc.tile_pool(name="sq_pool", bufs=2) as qpool,
                tc.tile_pool(name="sem_pool", bufs=NT) as spool,
            ):
                sems = []
                for nt in range(NT):            # 4 subchunks of 512 tokens
                    # one wide DMA: all 6 k-tiles of this subchunk
                    xk = xpool.tile([128, KT * 512], f32r, tag="xk")
                    nc.sync.dma_start(
                        out=xk.rearrange("p (k c) -> p k c", k=KT),
                        in_=bass.AP(
                            tensor=xT.tensor, offset=xT.offset + nt * 512,
                            ap=[[TOK, 128], [128 * TOK, KT], [1, 512]]))
                    psa = pA.tile([TC, 512], f32, tag="psa")
                    for k in range(KT):
                        nc.tensor.matmul(
                            psa,
                            lhsT=weff_sb[:, k * TC:(k + 1) * TC],
                            rhs=xk[:, k * 512:(k + 1) * 512],
                            start=(k == 0), stop=(k == KT - 1),
                        )
                    h2a = hapool.tile([TC, 512], f32, tag="h2a")
                    nc.vector.tensor_scalar_add(h2a, psa, beff_sb)
                    h2sq = qpool.tile([TC, 512], f32r, tag="h2sq")
                    nc.vector.tensor_mul(h2sq, h2a, h2a)
                    psq = pS.tile([CAP, 512], f32, tag="psq")
                    nc.tensor.matmul(psq, lhsT=sel_sb, rhs=h2sq,
                                     start=True, stop=True)
                    rt = qpool.tile([CAP, 512], f32, tag="rt")
                    nc.scalar.activation(rt, psq, Act.Sqrt)
                    den = qpool.tile([CAP, 512], f32, tag="den")
                    nc.vector.tensor_scalar_add(den, psq, 1.0)
                    rden = qpool.tile([CAP, 512], f32, tag="rden")
                    nc.vector.reciprocal(rden, den)
                    scal = qpool.tile([CAP, 512], f32r, tag="scal")
                    nc.vector.tensor_mul(scal, rt, rden)
                    prep = pR.tile([AC, 512], f32, tag="prep")
                    nc.tensor.matmul(prep, lhsT=rep_sb, rhs=scal,
                                     start=True, stop=True)
                    sem = spool.tile([AC, 512], f32, tag="sem")
                    nc.vector.tensor_tensor(out=sem, in0=h2a[:AC],
                                            in1=prep, op=Alu.mult)
                    sems.append(sem)
                # sem writes on the sync ring, after all x descriptors:
                # x drains first, each write fires as its DVE mul lands
                for nt in range(NT):
                    # rows (b_l, t, c), cols l: dims (tc, b, l) both sides
                    nc.sync.dma_start(
                        out=bass.AP(
                            tensor=sem_p[nt // 2].tensor,
                            offset=(sem_p[nt // 2].offset
                                    + (nt % 2) * 4 * AC * LCH),
                            ap=[[LCH, AC], [AC * LCH, 4], [1, LCH]]),
                        in_=sems[nt].rearrange("p (b l) -> p b l", b=4))
                    if nt % 2 == 1:
                        # fire this l-chunk's AllGather as soon as both
                        # subchunk writes land
                        nc.gpsimd.collective_compute(
                            "AllGather", Alu.bypass, replica_groups=RG,
                            ins=[sem_p[nt // 2][:]],
                            outs=[gath_p[nt // 2][:]])

            # ---- rw16 prefetch: on the sync ring after the x loads, so x
            # descriptors drain first and rw streams during the AllGather
            # window.  One DMA per (task, chunk): [128, CAP*L] fp16 ----
            rw16_sb = []
            with tc.tile_pool(name="rw16_pool", bufs=A) as rw16p:
                for k in range(NCH):
                    for r in range(A):
                        t16 = rw16p.tile([128, CAP * L], f16, tag="t16")
                        nc.sync.dma_start(
                            out=t16,
                            in_=rw16[:, (r * NCH + k) * CAP * L:
                                     (r * NCH + k + 1) * CAP * L])
                        rw16_sb.append((r, k, t16))

                # ===== Phase 2/3/4: per-chunk gather-transpose + priors ===
                # g tile: partition (ri, rank, b_l), cols (c', l_in_chunk);
                # PE transposes 128-col blocks (one per c') into semT; the
                # priors matmul accumulates over both chunks into pp[r].
                with (
                    tc.tile_pool(name="gpool", bufs=3) as gpool,
                    tc.tile_pool(name="pT", bufs=2, space="PSUM") as pT,
                    tc.tile_pool(name="pP", bufs=1, space="PSUM") as pP,
                    tc.tile_pool(name="rw32_pool", bufs=1) as rw32p,
                ):
                    pp = []
                    for r in range(A):
                        pp_r = pP.tile([64, L], f32, tag=f"pp{r}")
                        pp.append(pp_r)
                    ncast = 0
                    for k in range(NCH):
                        # upcast this chunk's rw to f32r (DVE/ACT split)
                        rw32 = {}
                        for (r, kk, t16) in rw16_sb:
                            if kk != k:
                                continue
                            t32 = rw32p.tile([128, CAP * L], f32r,
                                             tag=f"rw32_{r}")
                            if ncast % 2 == 0:
                                nc.vector.tensor_copy(out=t32, in_=t16)
                            else:
                                nc.scalar.activation(t32, t16, Act.Copy)
                            ncast += 1
                            rw32[r] = t32
                        for t in range(NPAIR):
                            nri = min(2, A - 2 * t)
                            g_sb = gpool.tile([128, CAP * LCH], f32, tag="g")
                            for ri in range(nri):
                                # partition (rank, b_l); per-partition read
                                # is 8 rows x 128 = 1KB contiguous
                                nc.sync.dma_start(
                                    out=g_sb[ri * 64:(ri + 1) * 64],
                                    in_=bass.AP(
                                        tensor=gath_p[k].tensor,
                                        offset=(gath_p[k].offset
                                                + (2 * t + ri) * CAP * LCH),
                                        ap=[[BL * AC * LCH, N_CORES],
                                            [AC * LCH, BL],
                                            [1, CAP * LCH]]))
                            blk0 = t * CAP
                            for j in range(CAP):
                                psT = pT.tile([128, 128], f32, tag="psT")
                                npart = nri * 64
                                nc.tensor.transpose(
                                    psT[:, :npart],
                                    in_=g_sb[:npart,
                                             j * LCH:(j + 1) * LCH],
                                    identity=ident_sb)
                                dst = semT_sb[:, (blk0 + j) * 128:
                                              (blk0 + j) * 128 + npart]
                                if j % 2 == 0:
                                    nc.vector.tensor_copy(out=dst,
                                                          in_=psT[:, :npart])
                                else:
                                    nc.scalar.activation(dst, psT[:, :npart],
                                                         Act.Copy)
                            for ri in range(nri):
                                r = 2 * t + ri
                                for j in range(CAP):
                                    base = (blk0 + j) * 128 + ri * 64
                                    nc.tensor.matmul(
                                        pp[r],
                                        lhsT=semT_sb[:, base:base + 64],
                                        rhs=rw32[r][:, j * L:(j + 1) * L],
                                        start=(k == 0 and j == 0),
                                        stop=(k == NCH - 1 and j == CAP - 1))
                        if k == NCH - 1:
                            for r in range(A):
                                dst = priors_sb[:, r * L:(r + 1) * L]
                                if r % 2 == 0:
                                    nc.vector.tensor_copy(out=dst, in_=pp[r])
                                else:
                                    nc.scalar.activation(dst, pp[r], Act.Copy)

            # ===== Phase 5: routing (folded scales) =======================
            with (
                tc.tile_pool(name="route", bufs=1) as rp,
                tc.tile_pool(name="pV", bufs=2, space="PSUM") as pV,
            ):
                big = rp.tile([64, A * L], f32)
                S = rp.tile([64, L], f32)
                v1 = rp.tile([64, L], f32)
                vote = rp.tile([64, L], f32)
                scr = rp.tile([64, A * L], f32)
                dots0 = rp.tile([64, A], f32)
                dots1 = rp.tile([64, A], f32)
                l1 = rp.tile([64, A], f32)
                l2 = rp.tile([64, A], f32)
                e1 = rp.tile([64, A], f32)
                e2 = rp.tile([64, A], f32)
                n2a = rp.tile([64, 1], f32)
                n2b = rp.tile([64, 1], f32)
                rta = rp.tile([64, 1], f32)
                rtb = rp.tile([64, 1], f32)
                dena = rp.tile([64, 1], f32)
                denb = rp.tile([64, 1], f32)
                rdena = rp.tile([64, 1], f32)
                rdenb = rp.tile([64, 1], f32)
                qa = rp.tile([64, 1], f32)
                qb = rp.tile([64, 1], f32)
                ssq = rp.tile([64, 1], f32)
                nmx = rp.tile([64, 1], f32)
                nmx2 = rp.tile([64, 1], f32)
                ssum1 = rp.tile([64, 1], f32)
                ssum2 = rp.tile([64, 1], f32)
                rsum2 = rp.tile([64, 1], f32)

                def warm(dep, m):
                    # tiny matmul with a true dep on the routing chain --
                    # keeps the PE HAM un-throttled through phase 5
                    pdum = pV.tile([64, 128], f32, tag="pdum")
                    nc.tensor.matmul(pdum[:m], lhsT=dep[:, 0:m],
                                     rhs=priors_sb[:, 0:128],
                                     start=True, stop=True)

                def bcast_r(t1):
                    # [64,1?]-free AP: broadcast [64, L] tile over r (A)
                    return bass.AP(tensor=t1.tensor, offset=t1.offset,
                                   ap=[t1.ap[0], [0, A], [1, L]])

                def bcast_o(tA):
                    # broadcast [64, A] tile over o (L)
                    return bass.AP(tensor=tA.tensor, offset=tA.offset,
                                   ap=[tA.ap[0], [1, A], [0, L]])

                p3 = priors_sb.rearrange("p (r o) -> p r o", r=A)

                # S = sum_r P_r  (strided reduce over r)
                nc.vector.tensor_reduce(
                    out=S, in_=priors_sb.rearrange("p (r o) -> p o r", r=A),
                    axis=X, op=Alu.add)
                # n2a = <S,S>
                nc.vector.scalar_tensor_tensor(
                    out=scr[:, 0:L], in0=S, scalar=1.0, in1=S,
                    op0=Alu.mult, op1=Alu.mult, accum_out=n2a)
                # dots0_r = <P_r, S>
                nc.vector.tensor_tensor(
                    out=big.rearrange("p (r o) -> p r o", r=A), in0=p3,
                    in1=bcast_r(S), op=Alu.mult)
                nc.vector.tensor_reduce(
                    out=dots0, in_=big.rearrange("p (r o) -> p r o", r=A),
                    axis=X, op=Alu.add)
                warm(S, 64)
                # qa = sqrt(n2a) / (A^2 + n2a)   ( = squash_scale(vote0)/A )
                nc.scalar.activation(rta, n2a, Act.Sqrt)
                nc.vector.tensor_scalar_add(dena, n2a, float(A * A))
                nc.vector.reciprocal(rdena, dena)
                nc.vector.tensor_mul(qa, rta, rdena)
                nc.vector.tensor_scalar_mul(l1, dots0, qa)
                # softmax(l1) unnormalized; ssum1 kept folded
                nc.vector.tensor_reduce(out=nmx, in_=l1, axis=X,
                                        op=Alu.max, negate=True)
                nc.scalar.activation(e1, l1, Act.Exp, bias=nmx,
                                     accum_out=ssum1)
                warm(l1, A)
                # v1 = sum_r e1_r P_r  (unnormalized vote1)
                nc.vector.tensor_tensor(
                    out=scr.rearrange("p (r o) -> p r o", r=A), in0=p3,
                    in1=bcast_o(e1), op=Alu.mult)
                nc.vector.tensor_reduce(
                    out=v1, in_=scr.rearrange("p (r o) -> p o r", r=A),
                    axis=X, op=Alu.add)
                # n2b = <v1,v1>; dots1_r = <P_r, v1>
                nc.vector.scalar_tensor_tensor(
                    out=scr[:, 0:L], in0=v1, scalar=1.0, in1=v1,
                    op0=Alu.mult, op1=Alu.mult, accum_out=n2b)
                nc.vector.tensor_tensor(
                    out=big.rearrange("p (r o) -> p r o", r=A), in0=p3,
                    in1=bcast_r(v1), op=Alu.mult)
                nc.vector.tensor_reduce(
                    out=dots1, in_=big.rearrange("p (r o) -> p r o", r=A),
                    axis=X, op=Alu.add)
                warm(v1, 64)
                # qb = sqrt(n2b) / (ssum1^2 + n2b)   ( = sc1/ssum1 )
                nc.scalar.activation(rtb, n2b, Act.Sqrt)
                nc.vector.tensor_mul(ssq, ssum1, ssum1)
                nc.vector.tensor_add(denb, ssq, n2b)
                nc.vector.reciprocal(rdenb, denb)
                nc.vector.tensor_mul(qb, rtb, rdenb)
                # l2 = l1 + qb * dots1
                nc.vector.scalar_tensor_tensor(
                    out=l2, in0=dots1, scalar=qb, in1=l1,
                    op0=Alu.mult, op1=Alu.add)
                nc.vector.tensor_reduce(out=nmx2, in_=l2, axis=X,
                                        op=Alu.max, negate=True)
                nc.scalar.activation(e2, l2, Act.Exp, bias=nmx2,
                                     accum_out=ssum2)
                nc.vector.reciprocal(rsum2, ssum2)
                warm(l2, A)
                # vote = sum_r (e2_r/ssum2) P_r   (final, not squashed);
                # the softmax normalization is folded into e2
                e2s = rp.tile([64, A], f32)
                nc.vector.tensor_scalar_mul(e2s, e2, rsum2)
                nc.vector.tensor_tensor(
                    out=scr.rearrange("p (r o) -> p r o", r=A), in0=p3,
                    in1=bcast_o(e2s), op=Alu.mult)
                nc.vector.tensor_reduce(
                    out=vote, in_=scr.rearrange("p (r o) -> p o r", r=A),
                    axis=X, op=Alu.add)

                # transpose vote [64, 256] -> voteT_dram [256, 64]
                vT_sb = rp.tile([128, 128], f32)
                for half in range(2):
                    pv = pV.tile([128, 64], f32, tag="pv")
                    nc.tensor.transpose(
                        pv, in_=vote[:, half * 128:(half + 1) * 128],
                        identity=ident_sb[:64, :64])
                    nc.vector.tensor_copy(
                        out=vT_sb[:, half * 64:(half + 1) * 64], in_=pv)
                    nc.sync.dma_start(
                        out=voteT_dram[half * 128:(half + 1) * 128],
                        in_=vT_sb[:, half * 64:(half + 1) * 64])

            # ===== Phase 6: final linear ==================================
            # voteT_dram[o, b]; vt2[cap, (lr, b)] loads with 256B bursts;
            # row CAP is all-ones so wlT9's bias row lands in the matmul.
            with (
                tc.tile_pool(name="vt", bufs=1) as vtp,
                tc.tile_pool(name="pF", bufs=4, space="PSUM") as pF,
                tc.tile_pool(name="outp", bufs=3) as op_,
            ):
                vt2 = vtp.tile([CAP + 1, 32 * B], f32)
                src = bass.AP(
                    tensor=voteT_dram.tensor, offset=voteT_dram.offset,
                    ap=[[B, CAP], [CAP * B, 32], [1, B]])
                nc.sync.dma_start(out=vt2[:CAP], in_=src)
                nc.sync.dma_start(out=vt2[CAP:CAP + 1], in_=ones_row)
                # permute free layout (lr, b) -> (b, lr) during the f32r
                # convert, so each lhsT is a contiguous 128-col slice
                vt2r = vtp.tile([CAP + 1, 32 * B], f32r)
                nc.vector.tensor_copy(
                    out=vt2r.rearrange("p (b lr) -> p b lr", lr=32),
                    in_=vt2.rearrange("p (lr b) -> p b lr", lr=32))
                NH = 2
                for b_l in range(BL):
                    for lt in range(2):
                        o_sb = op_.tile([128, D], f32, tag="o")
                        lhsT = vt2r[:, (b_l * CAP + lt * 4) * 32:
                                    (b_l * CAP + lt * 4) * 32 + 128]
                        for nh in range(NH):
                            pf = pF.tile([128, D // NH], f32, tag="pf")
                            nc.tensor.matmul(
                                pf, lhsT=lhsT,
                                rhs=wlT_sb[:, nh * (D // NH):
                                           (nh + 1) * (D // NH)],
                                start=True, stop=True)
                            dst = o_sb[:, nh * (D // NH):(nh + 1) * (D // NH)]
                            if nh == 0:
                                nc.vector.tensor_copy(out=dst, in_=pf)
                            else:
                                nc.scalar.activation(dst, pf, Act.Copy)
                        eng = nc.sync if (b_l % 2 == 0) else nc.scalar
                        eng.dma_start(
                            out=out[b_l, lt * 128:(lt + 1) * 128, :],
                            in_=o_sb)

    nc.compile()
    return nc


def _host_prep(x, fc1_w, fc1_b, fc2_w, fc2_b, route_weights, larger_w,
               larger_b, eval_t):
    A = int(eval_t) + 1
    f64 = np.float64
    weff = np.einsum("tcd,tdi->tci", fc2_w.astype(f64), fc1_w.astype(f64))
    beff = (np.einsum("tcd,td->tc", fc2_w.astype(f64), fc1_b.astype(f64))
            + fc2_b.astype(f64))
    weffT = np.ascontiguousarray(
        weff.reshape(NTASKS * CAP, D).T).astype(np.float32)
    beff_col = beff.reshape(NTASKS * CAP, 1).astype(np.float32)
    wlT9 = np.ascontiguousarray(np.concatenate(
        [larger_w[int(eval_t)].T, larger_b[int(eval_t)].reshape(1, D)],
        axis=0)).astype(np.float32)
    ones_row = np.ones((1, 32 * B), dtype=np.float32)
    selT = np.tile(np.eye(CAP, dtype=np.float32), (NTASKS, 1))
    repT = np.tile(np.eye(CAP, dtype=np.float32), (1, A))
    ident = np.eye(128, dtype=np.float32)

    in_maps = []
    for c in range(N_CORES):
        # x cols reordered to (chunk, b_l, l_in_chunk)
        xs = x[c * BL:(c + 1) * BL]                      # [8, 256, 768]
        xr = xs.reshape(BL, NCH, LCH, D).transpose(1, 0, 2, 3)
        xT_c = np.ascontiguousarray(
            xr.reshape(TOK, D).T).astype(np.float32)
        # rw16[p, ((r*NCH+k)*CAP+j)*L + o] =
        #   route_weights[c, r, (k*LCH+p)*CAP + j, o]
        rw2 = route_weights[c, :A].reshape(A, L, CAP, L)     # [r, l, c', o]
        rwt = rw2.reshape(A, NCH, LCH, CAP, L)               # [r, k, p, j, o]
        rw_c = np.ascontiguousarray(
            rwt.transpose(2, 0, 1, 3, 4).reshape(LCH, A * NCH * CAP * L)
        ).astype(np.float16)
        in_maps.append({
            "xT": xT_c, "weffT": weffT, "beff_col": beff_col, "rw16": rw_c,
            "wlT9": wlT9, "ones_row": ones_row, "selT": selT, "repT": repT,
            "ident": ident,
        })
    return A, in_maps


def kernel(**inputs):
    from concourse.bass_utils import run_bass_kernel_spmd

    A, in_maps = _host_prep(**inputs)
    if A not in _CACHE:
        _CACHE[A] = _build(A)
    nc = _CACHE[A]
    res = run_bass_kernel_spmd(nc, in_maps, core_ids=list(range(N_CORES)))
    return np.concatenate(
        [res.results[c]["out"] for c in range(N_CORES)], axis=0)


# revision 19
# speedup vs baseline: 1.2881x; 1.1158x over previous
"""CapsNet-BCL Trainium2 kernel: 8-core SPMD Bass/Tile implementation.

Host algebra: fc1/fc2 have no nonlinearity between them, so
Weff[t] = fc2_w[t] @ fc1_w[t], beff[t] = fc2_w[t]@fc1_b[t]+fc2_b[t] and
h2 = x @ Weff[t].T + beff[t].  Only tasks r <= eval_t route (softmax mask
-10000 underflows to exactly 0 in fp32), so only route_weights[:, :eval_t+1]
is read.

Sharding: core k computes h2/sem for batches [8k, 8k+8); sem is AllGathered
in two l-range chunks (l in [0,128) then [128,256)) so the collective
pipelines with phase 1; core c computes priors+routing for capsule c over
all 64 batches.  The torch flat view vote(CAP,B,1,L)->(B,L,CAP) maps output
batch b to vote capsule b//8, so core c's vote is exactly what output
batches [8c,8c+8) need: each core emits its own output slice.

Perf structure vs the previous baseline (228-265us):
 - phase 1 tokens are l-chunk-major: the sem AllGather for l-chunk 0 fires
   mid-phase-1 instead of after it, hiding most of the collective's
   ring latency behind compute; a tiny warm-up collective at t~0 absorbs
   the ncfw first-op reaction.
 - route_weights stored fp16 in DRAM (numerically validated: 5.5e-3 final
   rel err vs the 2e-2 gate) and upcast to f32r on DVE/ACT during the
   collective wait -- halves the dominant HBM stream; the priors matmul
   stays f32/f32r (sem quantization to fp16 was measured at 1.35e-2 --
   too close to the gate).
 - per-l-chunk gather-transpose + priors matmuls (PSUM accumulation over
   chunks) so only the last chunk's work sits on the tail.
 - routing rewritten with folded scales: squash norms and softmax
   normalizations are never materialized (q = sqrt(n2raw)/(A^2+n2raw)
   style foldings, verified algebraically equal); dot-batches via one
   broadcast multiply + segmented reduce.
 - x loaded in 4 wide DMAs (6 k-tiles each) to cut HWDGE issue time.
 - final-linear bias folded into the matmul as a 9th contraction row;
   output writes split across both HWDGE rings; small true-dep matmuls
   keep the PE HAM un-throttled through routing.
"""

import sys

import numpy as np

if "/opt/trn_rl_repo" not in sys.path:
    sys.path.insert(0, "/opt/trn_rl_repo")

NTASKS = 10
CAP = 8
L = 256
D = 768
B = 64
N_CORES = 8
BL = B // N_CORES          # batches per core (8)
TOK = BL * L               # tokens per core (2048)
KT = D // 128              # k tiles over D (6)
NCH = 2                    # l-chunks (asymmetric 3:1)
CHL = [192, 64]            # l per chunk
NBLK = [CAP * c // 128 for c in CHL]   # 128-wide i-blocks per chunk (12, 4)
NSUB = 4                   # subchunks per chunk (2 batches each)

_CACHE = {}


def _build(A):
    """Build the 8-core SPMD Bass program for A = eval_t+1 active tasks."""
    import concourse.bass as bass
    import concourse.tile as tile
    import concourse.mybir as mybir
    from concourse import bacc

    f32 = mybir.dt.float32
    f32r = mybir.dt.float32r
    f16 = mybir.dt.float16
    Alu = mybir.AluOpType
    Act = mybir.ActivationFunctionType
    X = mybir.AxisListType.X

    nc = bacc.Bacc("TRN2", target_bir_lowering=False, debug=False,
                   num_devices=N_CORES)

    TC = NTASKS * CAP  # 80
    AC = A * CAP       # active rows (48)
    NPAIR = (A + 1) // 2

    xT = nc.dram_tensor("xT", [D, TOK], f32r, kind="ExternalInput").ap()
    weffT = nc.dram_tensor("weffT", [D, TC], f32r, kind="ExternalInput").ap()
    beff_col = nc.dram_tensor("beff_col", [TC, 1], f32,
                              kind="ExternalInput").ap()
    # rw16: fp16, rows = i-block lane, cols (r, blk, o) where blk runs
    # over both chunks' 128-wide i-blocks (i = (c', l) c'-major per chunk)
    TBLK = sum(NBLK)  # 16
    rw16 = nc.dram_tensor("rw16", [128, A * TBLK * L], f16,
                          kind="ExternalInput").ap()
    # wlT9 = [larger_w[e].T; larger_b[e]] -- bias folded in as a 9th
    # contraction row; fp16 so the final matmul runs at 1 col/cycle
    wlT9 = nc.dram_tensor("wlT9", [CAP + 1, D], f16,
                          kind="ExternalInput").ap()
    ones_row = nc.dram_tensor("ones_row", [1, 32 * B], f32,
                              kind="ExternalInput").ap()
    # squash helpers: selT[(t,c), c'] = (c == c'); repT[c, (t<A,c')] = (c==c')
    selT = nc.dram_tensor("selT", [TC, CAP], f32r,
                          kind="ExternalInput").ap()
    repT = nc.dram_tensor("repT", [CAP, AC], f32r,
                          kind="ExternalInput").ap()
    ident = nc.dram_tensor("ident", [128, 128], f32, kind="ExternalInput").ap()
    out = nc.dram_tensor("out", [BL, L, D], f32, kind="ExternalOutput").ap()

    # collective chunks by l-range: rows (b_l, t, c), cols l_in_chunk
    # (b_l outermost so the post-gather read is contiguous per batch)
    sem_p = [nc.dram_tensor(f"sem_p{i}", [BL * AC, CHL[i]], f32).ap()
             for i in range(NCH)]
    gath_p = [nc.dram_tensor(f"gath_p{i}", [N_CORES * BL * AC, CHL[i]], f32,
                             addr_space="Shared").ap()
              for i in range(NCH)]
    # tiny collective fired at t~0 with no deps: wakes ncfw / absorbs the
    # first-op reaction cost while phase 1 is still computing
    cc_warm_in = nc.dram_tensor("cc_warm_in", [1, 16], f32).ap()
    cc_warm_out = nc.dram_tensor("cc_warm_out", [N_CORES, 16], f32,
                                 addr_space="Shared").ap()
    voteT_dram = nc.dram_tensor("voteT_dram", [L, B], f32).ap()

    RG = [list(range(N_CORES))]

    with tile.TileContext(nc) as tc:
        with tc.tile_pool(name="singles", bufs=1) as singles:
            # warm-up collective: fires as soon as its 64B input is zeroed
            # (collectives cannot read IO tensors, so memset+DMA first)
            warm_sb = singles.tile([1, 16], f32)
            nc.vector.memset(warm_sb, 0.0)
            nc.gpsimd.dma_start(out=cc_warm_in, in_=warm_sb)
            nc.gpsimd.collective_compute(
                "AllGather", Alu.bypass, replica_groups=RG,
                ins=[cc_warm_in[:]], outs=[cc_warm_out[:]])

            # ---- constants (scalar ring; sync ring is reserved for x) ----
            weff_sb = singles.tile([128, KT * TC], f32r)
            nc.scalar.dma_start(out=weff_sb,
                                in_=weffT.rearrange("(k p) c -> p k c", p=128))
            beff_sb = singles.tile([TC, 1], f32)
            nc.scalar.dma_start(out=beff_sb, in_=beff_col)
            ident_sb = singles.tile([128, 128], f32)
            nc.scalar.dma_start(out=ident_sb, in_=ident)
            wlT_sb = singles.tile([CAP + 1, D], f16)
            nc.scalar.dma_start(out=wlT_sb, in_=wlT9)
            sel_sb = singles.tile([TC, CAP], f32r)
            nc.scalar.dma_start(out=sel_sb, in_=selT)
            rep_sb = singles.tile([CAP, AC], f32r)
            nc.scalar.dma_start(out=rep_sb, in_=repT)

            priors_sb = singles.tile([64, A * L], f32)
            # semT[(i-slice), (pair block, (ri, rank, b_l))] -- reused
            # across chunks (PE FIFO order makes the WAR free)
            semT_sb = singles.tile([128, NPAIR * NBLK[0] * 128], f32r)

            # preload the ACT Sqrt table before phase 1 needs it
            sqrt_warm = singles.tile([1, 16], f32)
            nc.scalar.activation(sqrt_warm, ident_sb[0:1, 0:16], Act.Sqrt)

            # ===== Phase 1: semantic stage, batch-parallel ================
            # cols are (chunk, b_l, l_in_chunk); per 512-col subchunk:
            #   h2a = psa + beff (ACT, bias); h2sq = h2a^2 (DVE);
            #   sq[c, tok] = selT.T @ h2sq (PE); scal = sqrt(sq)/(1+sq)
            #   via ACT Sqrt + DVE reciprocal; scal_rep = repT.T @ scal (PE);
            #   sem = h2a[:AC] * scal_rep (DVE) -> DRAM chunk buffer.
            with (
                tc.tile_pool(name="x_pool", bufs=2) as xpool,
                tc.tile_pool(name="pA", bufs=1, space="PSUM") as pA,
                tc.tile_pool(name="pS", bufs=1, space="PSUM") as pS,
                tc.tile_pool(name="pR", bufs=1, space="PSUM") as pR,
                tc.tile_pool(name="h2a_pool", bufs=2) as hapool,
                tc.tile_pool(name="sq_pool", bufs=2) as qpool,
                tc.tile_pool(name="sem_pool", bufs=NSUB) as spool,
            ):
                sems = []
                for k in range(NCH):
                    W = 2 * CHL[k]          # subchunk width (2 batches)
                    cbase = BL * CHL[0] if k else 0
                    for s in range(NSUB):
                        xk = xpool.tile([128, KT * W], f32r, tag=f"xk{k}")
                        nc.sync.dma_start(
                            out=xk.rearrange("p (kk c) -> p kk c", kk=KT),
                            in_=bass.AP(
                                tensor=xT.tensor,
                                offset=xT.offset + cbase + s * W,
                                ap=[[TOK, 128], [128 * TOK, KT], [1, W]]))
                        psa = pA.tile([TC, W], f32, tag=f"psa{k}")
                        for kk in range(KT):
                            nc.tensor.matmul(
                                psa,
                                lhsT=weff_sb[:, kk * TC:(kk + 1) * TC],
                                rhs=xk[:, kk * W:(kk + 1) * W],
                                start=(kk == 0), stop=(kk == KT - 1),
                            )
                        h2a = hapool.tile([TC, W], f32, tag=f"h2a{k}")
                        nc.vector.tensor_scalar_add(h2a, psa, beff_sb)
                        h2sq = qpool.tile([TC, W], f32r, tag=f"h2sq{k}")
                        nc.vector.tensor_mul(h2sq, h2a, h2a)
                        psq = pS.tile([CAP, W], f32, tag=f"psq{k}")
                        nc.tensor.matmul(psq, lhsT=sel_sb, rhs=h2sq,
                                         start=True, stop=True)
                        rt = qpool.tile([CAP, W], f32, tag=f"rt{k}")
                        nc.scalar.activation(rt, psq, Act.Sqrt)
                        den = qpool.tile([CAP, W], f32, tag=f"den{k}")
                        nc.vector.tensor_scalar_add(den, psq, 1.0)
                        rden = qpool.tile([CAP, W], f32, tag=f"rden{k}")
                        nc.vector.reciprocal(rden, den)
                        scal = qpool.tile([CAP, W], f32r, tag=f"scal{k}")
                        nc.vector.tensor_mul(scal, rt, rden)
                        prep = pR.tile([AC, W], f32, tag=f"prep{k}")
                        nc.tensor.matmul(prep, lhsT=rep_sb, rhs=scal,
                                         start=True, stop=True)
                        sem = spool.tile([AC, W], f32, tag=f"sem{k}")
                        nc.vector.tensor_tensor(out=sem, in0=h2a[:AC],
                                                in1=prep, op=Alu.mult)
                        sems.append((k, s, sem))
                # sem writes on the sync ring, after all x descriptors:
                # x drains first, each write fires as its DVE mul lands
                for (k, s, sem) in sems:
                    # rows (b_l, t, c), cols l: dims (tc, b, l) both sides
                    nc.sync.dma_start(
                        out=bass.AP(
                            tensor=sem_p[k].tensor,
                            offset=(sem_p[k].offset
                                    + s * 2 * AC * CHL[k]),
                            ap=[[CHL[k], AC], [AC * CHL[k], 2],
                                [1, CHL[k]]]),
                        in_=sem.rearrange("p (b l) -> p b l", b=2))
                    if s == NSUB - 1:
                        # fire this l-chunk's AllGather as soon as all
                        # subchunk writes land
                        nc.gpsimd.collective_compute(
                            "AllGather", Alu.bypass, replica_groups=RG,
                            ins=[sem_p[k][:]],
                            outs=[gath_p[k][:]])

            # ---- rw16 prefetch: on the sync ring after the x loads, so x
            # descriptors drain first and rw streams during the AllGather
            # window.  One DMA per (task, chunk): [128, CAP*L] fp16 ----
            rw16_sb = []
            with tc.tile_pool(name="rw16_pool", bufs=3) as rw16p:
                for k in range(NCH):
                    bb = 0 if k == 0 else NBLK[0]
                    for r in range(A):
                        t16 = rw16p.tile([128, NBLK[k] * L], f16,
                                         tag=f"t16_{k}")
                        nc.sync.dma_start(
                            out=t16,
                            in_=rw16[:, (r * TBLK + bb) * L:
                                     (r * TBLK + bb + NBLK[k]) * L])
                        rw16_sb.append((r, k, t16))

                # ===== Phase 2/3/4: per-chunk gather-transpose + priors ===
                # g tile: partition (ri, rank, b_l), cols (c', l_in_chunk);
                # PE transposes 128-col blocks (one per c') into semT; the
                # priors matmul accumulates over both chunks into pp[r].
                with (
                    tc.tile_pool(name="gpool", bufs=3) as gpool,
                    tc.tile_pool(name="pT", bufs=2, space="PSUM") as pT,
                    tc.tile_pool(name="pP", bufs=1, space="PSUM") as pP,
                    tc.tile_pool(name="rw32_pool", bufs=1) as rw32p,
                ):
                    pp = []
                    for r in range(A):
                        pp_r = pP.tile([64, L], f32, tag=f"pp{r}")
                        pp.append(pp_r)
                    ncast = 0
                    for k in range(NCH):
                        # upcast this chunk's rw to f32r (DVE/ACT split)
                        rw32 = {}
                        for (r, kk, t16) in rw16_sb:
                            if kk != k:
                                continue
                            t32 = rw32p.tile([128, NBLK[k] * L], f32r,
                                             tag=f"rw32_{r}_{k}")
                            if ncast % 2 == 0:
                                nc.vector.tensor_copy(out=t32, in_=t16)
                            else:
                                nc.scalar.activation(t32, t16, Act.Copy)
                            ncast += 1
                            rw32[r] = t32
                        CL = CHL[k]
                        for t in range(NPAIR):
                            nri = min(2, A - 2 * t)
                            g_sb = gpool.tile([128, CAP * CL], f32,
                                              tag=f"g{k}")
                            for ri in range(nri):
                                # partition (rank, b_l); per-partition read
                                # is 8 rows x CL contiguous
                                nc.sync.dma_start(
                                    out=g_sb[ri * 64:(ri + 1) * 64],
                                    in_=bass.AP(
                                        tensor=gath_p[k].tensor,
                                        offset=(gath_p[k].offset
                                                + (2 * t + ri) * CAP * CL),
                                        ap=[[BL * AC * CL, N_CORES],
                                            [AC * CL, BL],
                                            [1, CAP * CL]]))
                            for j in range(NBLK[k]):
                                psT = pT.tile([128, 128], f32, tag="psT")
                                npart = nri * 64
                                nc.tensor.transpose(
                                    psT[:, :npart],
                                    in_=g_sb[:npart,
                                             j * 128:(j + 1) * 128],
                                    identity=ident_sb)
                                dst = semT_sb[:, (t * NBLK[0] + j) * 128:
                                              (t * NBLK[0] + j) * 128 + npart]
                                if j % 2 == 0:
                                    nc.vector.tensor_copy(out=dst,
                                                          in_=psT[:, :npart])
                                else:
                                    nc.scalar.activation(dst, psT[:, :npart],
                                                         Act.Copy)
                            for ri in range(nri):
                                r = 2 * t + ri
                                for j in range(NBLK[k]):
                                    base = (t * NBLK[0] + j) * 128 + ri * 64
                                    nc.tensor.matmul(
                                        pp[r],
                                        lhsT=semT_sb[:, base:base + 64],
                                        rhs=rw32[r][:, j * L:(j + 1) * L],
                                        start=(k == 0 and j == 0),
                                        stop=(k == NCH - 1
                                              and j == NBLK[k] - 1))
                        if k == NCH - 1:
                            for r in range(A):
                                dst = priors_sb[:, r * L:(r + 1) * L]
                                if r % 2 == 0:
                                    nc.vector.tensor_copy(out=dst, in_=pp[r])
                                else:
                                    nc.scalar.activation(dst, pp[r], Act.Copy)

            # ===== Phase 5: routing (folded scales) =======================
            with (
                tc.tile_pool(name="route", bufs=1) as rp,
                tc.tile_pool(name="pV", bufs=2, space="PSUM") as pV,
            ):
                big = rp.tile([64, A * L], f32)
                S = rp.tile([64, L], f32)
                v1 = rp.tile([64, L], f32)
                vote = rp.tile([64, L], f32)
                scr = rp.tile([64, A * L], f32)
                dots0 = rp.tile([64, A], f32)
                dots1 = rp.tile([64, A], f32)
                l1 = rp.tile([64, A], f32)
                l2 = rp.tile([64, A], f32)
                e1 = rp.tile([64, A], f32)
                e2 = rp.tile([64, A], f32)
                n2a = rp.tile([64, 1], f32)
                n2b = rp.tile([64, 1], f32)
                rta = rp.tile([64, 1], f32)
                rtb = rp.tile([64, 1], f32)
                dena = rp.tile([64, 1], f32)
                denb = rp.tile([64, 1], f32)
                rdena = rp.tile([64, 1], f32)
                rdenb = rp.tile([64, 1], f32)
                qa = rp.tile([64, 1], f32)
                qb = rp.tile([64, 1], f32)
                ssq = rp.tile([64, 1], f32)
                nmx = rp.tile([64, 1], f32)
                nmx2 = rp.tile([64, 1], f32)
                ssum1 = rp.tile([64, 1], f32)
                ssum2 = rp.tile([64, 1], f32)
                rsum2 = rp.tile([64, 1], f32)

                def warm(dep, m):
                    # tiny matmul with a true dep on the routing chain --
                    # keeps the PE HAM un-throttled through phase 5
                    pdum = pV.tile([64, 128], f32, tag="pdum")
                    nc.tensor.matmul(pdum[:m], lhsT=dep[:, 0:m],
                                     rhs=priors_sb[:, 0:128],
                                     start=True, stop=True)

                def bcast_r(t1):
                    # [64,1?]-free AP: broadcast [64, L] tile over r (A)
                    return bass.AP(tensor=t1.tensor, offset=t1.offset,
                                   ap=[t1.ap[0], [0, A], [1, L]])

                def bcast_o(tA):
                    # broadcast [64, A] tile over o (L)
                    return bass.AP(tensor=tA.tensor, offset=tA.offset,
                                   ap=[tA.ap[0], [1, A], [0, L]])

                p3 = priors_sb.rearrange("p (r o) -> p r o", r=A)

                # S = sum_r P_r  (strided reduce over r)
                nc.vector.tensor_reduce(
                    out=S, in_=priors_sb.rearrange("p (r o) -> p o r", r=A),
                    axis=X, op=Alu.add)
                # n2a = <S,S>
                nc.vector.scalar_tensor_tensor(
                    out=scr[:, 0:L], in0=S, scalar=1.0, in1=S,
                    op0=Alu.mult, op1=Alu.mult, accum_out=n2a)
                # dots0_r = <P_r, S>
                nc.vector.tensor_tensor(
                    out=big.rearrange("p (r o) -> p r o", r=A), in0=p3,
                    in1=bcast_r(S), op=Alu.mult)
                nc.vector.tensor_reduce(
                    out=dots0, in_=big.rearrange("p (r o) -> p r o", r=A),
                    axis=X, op=Alu.add)
                warm(S, 64)
                # qa = sqrt(n2a) / (A^2 + n2a)   ( = squash_scale(vote0)/A )
                nc.scalar.activation(rta, n2a, Act.Sqrt)
                nc.vector.tensor_scalar_add(dena, n2a, float(A * A))
                nc.vector.reciprocal(rdena, dena)
                nc.vector.tensor_mul(qa, rta, rdena)
                nc.vector.tensor_scalar_mul(l1, dots0, qa)
                # softmax(l1) unnormalized; ssum1 kept folded
                nc.vector.tensor_reduce(out=nmx, in_=l1, axis=X,
                                        op=Alu.max, negate=True)
                nc.scalar.activation(e1, l1, Act.Exp, bias=nmx,
                                     accum_out=ssum1)
                warm(l1, A)
                # v1 = sum_r e1_r P_r  (unnormalized vote1)
                nc.vector.tensor_tensor(
                    out=scr.rearrange("p (r o) -> p r o", r=A), in0=p3,
                    in1=bcast_o(e1), op=Alu.mult)
                nc.vector.tensor_reduce(
                    out=v1, in_=scr.rearrange("p (r o) -> p o r", r=A),
                    axis=X, op=Alu.add)
                # n2b = <v1,v1>; dots1_r = <P_r, v1>
                nc.vector.scalar_tensor_tensor(
                    out=scr[:, 0:L], in0=v1, scalar=1.0, in1=v1,
                    op0=Alu.mult, op1=Alu.mult, accum_out=n2b)
                nc.vector.tensor_tensor(
                    out=big.rearrange("p (r o) -> p r o", r=A), in0=p3,
                    in1=bcast_r(v1), op=Alu.mult)
                nc.vector.tensor_reduce(
                    out=dots1, in_=big.rearrange("p (r o) -> p r o", r=A),
                    axis=X, op=Alu.add)
                warm(v1, 64)
                # qb = sqrt(n2b) / (ssum1^2 + n2b)   ( = sc1/ssum1 )
                nc.scalar.activation(rtb, n2b, Act.Sqrt)
                nc.vector.tensor_mul(ssq, ssum1, ssum1)
                nc.vector.tensor_add(denb, ssq, n2b)
                nc.vector.reciprocal(rdenb, denb)
                nc.vector.tensor_mul(qb, rtb, rdenb)
                # l2 = l1 + qb * dots1
                nc.vector.scalar_tensor_tensor(
                    out=l2, in0=dots1, scalar=qb, in1=l1,
                    op0=Alu.mult, op1=Alu.add)
                nc.vector.tensor_reduce(out=nmx2, in_=l2, axis=X,
                                        op=Alu.max, negate=True)
                nc.scalar.activation(e2, l2, Act.Exp, bias=nmx2,
                                     accum_out=ssum2)
                nc.vector.reciprocal(rsum2, ssum2)
                warm(l2, A)
                # vote = sum_r (e2_r/ssum2) P_r   (final, not squashed);
                # the softmax normalization is folded into e2
                e2s = rp.tile([64, A], f32)
                nc.vector.tensor_scalar_mul(e2s, e2, rsum2)
                nc.vector.tensor_tensor(
                    out=scr.rearrange("p (r o) -> p r o", r=A), in0=p3,
                    in1=bcast_o(e2s), op=Alu.mult)
                nc.vector.tensor_reduce(
                    out=vote, in_=scr.rearrange("p (r o) -> p o r", r=A),
                    axis=X, op=Alu.add)

                # transpose vote [64, 256] -> voteT_dram [256, 64]
                vT_sb = rp.tile([128, 128], f32)
                for half in range(2):
                    pv = pV.tile([128, 64], f32, tag="pv")
                    nc.tensor.transpose(
                        pv, in_=vote[:, half * 128:(half + 1) * 128],
                        identity=ident_sb[:64, :64])
                    nc.vector.tensor_copy(
                        out=vT_sb[:, half * 64:(half + 1) * 64], in_=pv)
                    nc.sync.dma_start(
                        out=voteT_dram[half * 128:(half + 1) * 128],
                        in_=vT_sb[:, half * 64:(half + 1) * 64])

            # ===== Phase 6: final linear ==================================
            # voteT_dram[o, b]; vt2[cap, (lr, b)] loads with 256B bursts;
            # row CAP is all-ones so wlT9's bias row lands in the matmul.
            with (
                tc.tile_pool(name="vt", bufs=1) as vtp,
                tc.tile_pool(name="pF", bufs=4, space="PSUM") as pF,
                tc.tile_pool(name="outp", bufs=3) as op_,
            ):
                vt2 = vtp.tile([CAP + 1, 32 * B], f32)
                src = bass.AP(
                    tensor=voteT_dram.tensor, offset=voteT_dram.offset,
                    ap=[[B, CAP], [CAP * B, 32], [1, B]])
                nc.sync.dma_start(out=vt2[:CAP], in_=src)
                nc.sync.dma_start(out=vt2[CAP:CAP + 1], in_=ones_row)
                # permute free layout (lr, b) -> (b, lr) during the f32r
                # convert, so each lhsT is a contiguous 128-col slice
                vt2r = vtp.tile([CAP + 1, 32 * B], f16)
                nc.vector.tensor_copy(
                    out=vt2r.rearrange("p (b lr) -> p b lr", lr=32),
                    in_=vt2.rearrange("p (lr b) -> p b lr", lr=32))
                NH = 2
                for b_l in range(BL):
                    for lt in range(2):
                        o_sb = op_.tile([128, D], f32, tag="o")
                        lhsT = vt2r[:, (b_l * CAP + lt * 4) * 32:
                                    (b_l * CAP + lt * 4) * 32 + 128]
                        for nh in range(NH):
                            pf = pF.tile([128, D // NH], f32, tag="pf")
                            nc.tensor.matmul(
                                pf, lhsT=lhsT,
                                rhs=wlT_sb[:, nh * (D // NH):
                                           (nh + 1) * (D // NH)],
                                start=True, stop=True)
                            dst = o_sb[:, nh * (D // NH):(nh + 1) * (D // NH)]
                            if nh == 0:
                                nc.vector.tensor_copy(out=dst, in_=pf)
                            else:
                                nc.scalar.activation(dst, pf, Act.Copy)
                        eng = nc.sync if (b_l % 2 == 0) else nc.scalar
                        eng.dma_start(
                            out=out[b_l, lt * 128:(lt + 1) * 128, :],
                            in_=o_sb)

    nc.compile()
    return nc


def _host_prep(x, fc1_w, fc1_b, fc2_w, fc2_b, route_weights, larger_w,
               larger_b, eval_t):
    A = int(eval_t) + 1
    f64 = np.float64
    weff = np.einsum("tcd,tdi->tci", fc2_w.astype(f64), fc1_w.astype(f64))
    beff = (np.einsum("tcd,td->tc", fc2_w.astype(f64), fc1_b.astype(f64))
            + fc2_b.astype(f64))
    weffT = np.ascontiguousarray(
        weff.reshape(NTASKS * CAP, D).T).astype(np.float32)
    beff_col = beff.reshape(NTASKS * CAP, 1).astype(np.float32)
    wlT9 = np.ascontiguousarray(np.concatenate(
        [larger_w[int(eval_t)].T, larger_b[int(eval_t)].reshape(1, D)],
        axis=0)).astype(np.float16)
    ones_row = np.ones((1, 32 * B), dtype=np.float32)
    selT = np.tile(np.eye(CAP, dtype=np.float32), (NTASKS, 1))
    repT = np.tile(np.eye(CAP, dtype=np.float32), (1, A))
    ident = np.eye(128, dtype=np.float32)

    in_maps = []
    for c in range(N_CORES):
        # x cols reordered to (chunk, b_l, l_in_chunk), chunks 192/64
        xs = x[c * BL:(c + 1) * BL]                      # [8, 256, 768]
        parts = [xs[:, :CHL[0], :].reshape(BL * CHL[0], D),
                 xs[:, CHL[0]:, :].reshape(BL * CHL[1], D)]
        xT_c = np.ascontiguousarray(
            np.concatenate(parts, axis=0).T).astype(np.float32)
        # rw16 cols (r, blk, o); blk = 128-wide slices of the per-chunk
        # (c'-major, l_in_chunk) flattened contraction order
        rw2 = route_weights[c, :A].reshape(A, L, CAP, L)     # [r, l, c', o]
        chunks = []
        lb = 0
        for k in range(NCH):
            cc = rw2[:, lb:lb + CHL[k]]                      # [r, l, c', o]
            lb += CHL[k]
            cc = cc.transpose(0, 2, 1, 3).reshape(A, NBLK[k], 128, L)
            chunks.append(cc)
        rw_all = np.concatenate(chunks, axis=1)              # [r, TBLK,128,L]
        rw_c = np.ascontiguousarray(
            rw_all.transpose(2, 0, 1, 3).reshape(128, A * sum(NBLK) * L)
        ).astype(np.float16)
        in_maps.append({
            "xT": xT_c, "weffT": weffT, "beff_col": beff_col, "rw16": rw_c,
            "wlT9": wlT9, "ones_row": ones_row, "selT": selT, "repT": repT,
            "ident": ident,
        })
    return A, in_maps


def kernel(**inputs):
    from concourse.bass_utils import run_bass_kernel_spmd

    A, in_maps = _host_prep(**inputs)
    if A not in _CACHE:
        _CACHE[A] = _build(A)
    nc = _CACHE[A]
    res = run_bass_kernel_spmd(nc, in_maps, core_ids=list(range(N_CORES)))
    return np.concatenate(
        [res.results[c]["out"] for c in range(N_CORES)], axis=0)
